# revision 1
# baseline (speedup 1.0000x reference)
"""Deformable conv (DCNv2) Bass kernel for trn2, data-parallel over batch on 8 cores.

Per-core pipeline (one batch sample per NeuronCore):
  1. x -> SBUF as bf16 "adjacent-pair table" xe[p, 2i]=xpad[i], xe[p,2i+1]=xpad[i+1]
     (zero-padded image, 1 row top/bot, 2 cols l/r), duplicated on partitions 64-127.
  2. offset/mask 3x3 convs as 9 shifted matmuls + a "ramp" matmul folding the
     h/w base grid; ACT adds bias (+ tap const) and sigmoids the mask.
  3. fp32 DVE chain: floor, frac, clamps -> bilinear corner scales (mask-folded,
     bf16, (left,right)-interleaved) and flat gather indices.
  4. index wrap for ap_gather built with PE transposes + constant permutation
     matmuls, converted to int16.
  5. main loop, software-pipelined with the preamble: pass p produces
     exactly gather-window p's indices, and pass p+1's preamble is emitted
     after window p's first tap-pair, so its conv/chain/wrap fills PE/DVE/
     ACT (and, via FIFO pool order, gets PSUM slots) while Pool chews the
     remaining gathers. Per (window, tap-pair): one GPSIMD ap_gather (d=2
     bf16 pairs; 2 taps via partition groups; top+bottom rows merged), PE
     scale-broadcast matmuls, DVE modulated multiply (bf16 2x), 4 corner
     matmuls accumulating out[o,j] in PSUM across the 5 tap-pairs, DMA out.

Timeline model, per core: span ~2.0ms fully gather-bound (40 ap_gather
calls back-to-back, 69us total Pool idle = startup only). The model
overcharges ap_gather by billing its whole source AP per call; under a
realistic ~11us/call gather cost the same schedule spans ~0.69ms with
all engines balanced (Pool 0.46 / PE 0.41 / ACT 0.39 / DVE 0.35ms busy),
so the real span is bounded by [~0.69ms, ~2.0ms]. Cores batch-parallel.

Landed: pair-4 gather de-duplication (-10% Pool work): the tap-8 call
splits the window's positions across the two partition-group halves
instead of duplicating the stream (num_idxs 4096->2048); its second half
is consumed via base-64 K=64 matmuls (tap8-only lhsT block) with extra
SELBC columns, and the wrap converts route slot-quadrants per half.

Analyzed-but-rejected (do not retry without real NTFF profiling):
- Conv tap-pairing via a host-shifted upper xe copy (-31us PE busy) was
  implemented and measured: it regressed the end-to-end span in both
  gather-cost regimes (+16/+26us) because PE is not the binding engine
  and the split index-convert lengthened the IDXW critical path. Reverted;
  don't retry without real NTFF profiling.
- d=4 single-index 4-corner gather is infeasible: ap_gather indexes
  d-element units (not elements), forcing a 4x-redundant 139KB/partition
  table; also exceeds the int16 AP-step delta range.
"""
import sys

for _p in ("/opt/trn_rl_repo", "/opt/pypackages"):
    if _p not in sys.path:
        sys.path.append(_p)

import numpy as np
import ml_dtypes

BF16 = ml_dtypes.bfloat16

B, C, H, W = 8, 64, 128, 128
OUT, K = 128, 9
NCORES = 8
NPAIR = 5  # 4 real tap pairs + (tap8, dup-tap8-with-zero-weights)


def _params(h, w):
    hw = h * w
    d = dict(H=h, W=w, HW=hw, PH=h + 2, PW=w + 4, NCH=hw // 512,
             NPASS=max(1, min(8, (hw // 512) // 4)), NG=4,
             GCH=2048 if hw >= 2048 else hw, RPC=512 // w)
    d["NE"] = d["PH"] * d["PW"]
    d["QW"] = hw // d["NG"] // d["NPASS"]
    d["CPP"] = d["NCH"] // d["NPASS"]
    return d


def _tap_of(pair, half):
    t = 2 * pair + half
    return 8 if t > 8 else t


def build_xe(x, h=H, w=W):
    """bf16 adjacent-pair table of the zero-padded image: [C, 2*NE]."""
    P = _params(h, w)
    PH, PW, NE = P["PH"], P["PW"], P["NE"]
    xpad = np.zeros((C, PH, PW), np.float32)
    xpad[:, 1:1 + h, 2:2 + w] = x
    flat = np.concatenate([xpad.reshape(C, NE),
                           np.zeros((C, 1), np.float32)], axis=1)
    xe = np.stack([flat[:, :NE], flat[:, 1:NE + 1]], axis=-1)  # [C, NE, 2]
    return xe.reshape(C, 2 * NE).astype(BF16)


def host_consts(w_offset, b_offset, w_mask, b_mask, w_conv, h=H, w=W):
    P = _params(h, w)
    ky = np.repeat(np.arange(3), 3).astype(np.int64)
    kx = np.tile(np.arange(3), 3).astype(np.int64)

    # conv output rows padded to quadrant bases: gy 0-8, gx 32-40, m 64-72
    WOM = np.zeros((C, 9 * 96), np.float32)
    for t in range(9):
        for k in range(9):
            WOM[:, 96 * t + k] = w_offset[2 * k, :, ky[t], kx[t]]
            WOM[:, 96 * t + 32 + k] = w_offset[2 * k + 1, :, ky[t], kx[t]]
            WOM[:, 96 * t + 64 + k] = w_mask[k, :, ky[t], kx[t]]

    RL = np.zeros((3, P["NCH"] * 96), np.float32)
    for c in range(P["NCH"]):
        RL[0, 96 * c: 96 * c + 9] = float(c * P["RPC"])  # gy += h0
        RL[1, 96 * c: 96 * c + 9] = 1.0                  # gy += hsub
        RL[2, 96 * c + 32: 96 * c + 41] = 1.0            # gx += wsub
    j = np.arange(512)
    R3 = np.stack([np.ones(512, np.float32),
                   (j // w).astype(np.float32),
                   (j % w).astype(np.float32)])

    BGY = (b_offset[0::2] + ky - 1.0).astype(np.float32).reshape(9, 1)
    BGX = (b_offset[1::2] + kx - 1.0).astype(np.float32).reshape(9, 1)
    BM = b_mask.astype(np.float32).reshape(9, 1)

    WCONV = np.zeros((128, (NPAIR + 1) * 128), np.float32)
    wc3 = w_conv.reshape(OUT, C, 9)
    for p in range(NPAIR):
        for half in range(2):
            t = 2 * p + half
            if t > 8:
                continue
            WCONV[half * 64:half * 64 + 64, 128 * p:128 * p + 128] = wc3[:, :, t].T
    WCONV[64:128, 128 * NPAIR:128 * (NPAIR + 1)] = wc3[:, :, 8].T
    IDENT = np.eye(128, dtype=np.float32)
    SEL = np.zeros((128, 8 * 128), np.float32)
    for b_ in range(8):
        for qp in range(128):
            SEL[16 * b_ + qp % 16, 128 * b_ + qp] = 1.0
    # broadcast-select: for (pair, group) pick scale rows {9r+2p (cols 0-63),
    # 9r+2p+1 (cols 64-127)} out of the [40, N] scale tensor
    SELBC = np.zeros((128, 24 * 128), np.float32)
    for p in range(NPAIR):
        for r in range(4):
            base = 128 * (4 * p + r)
            SELBC[32 * r + 2 * p, base:base + 64] = 1.0
            SELBC[32 * r + 2 * p + 1, base + 64:base + 128] = 1.0
    for r in range(4):
        base = 128 * (20 + r)
        SELBC[32 * r + 8, base + 64:base + 128] = 1.0
    return {
        "wom": WOM.astype(BF16), "rl": RL.astype(BF16), "r3": R3.astype(BF16),
        "bgy": BGY, "bgx": BGX, "bm": BM,
        "wconv": WCONV.astype(BF16), "ident": IDENT, "sel": SEL,
        "selbc": SELBC.astype(BF16),
    }


def emit(nc, tc, mybir, dram, h=H, w=W):
    P = _params(h, w)
    HW, PH, PW, NE = P["HW"], P["PH"], P["PW"], P["NE"]
    NCH, NPASS, QW, GCH, RPC, CPP = (P["NCH"], P["NPASS"], P["QW"], P["GCH"],
                                     P["RPC"], P["CPP"])
    f32, bf16, i16 = mybir.dt.float32, mybir.dt.bfloat16, mybir.dt.int16
    AF = mybir.ActivationFunctionType
    OP = mybir.AluOpType
    MAGIC = 12582912.0  # 1.5 * 2^23: fp32 round-to-nearest-int trick

    from contextlib import ExitStack
    ctx = ExitStack()
    sbC = ctx.enter_context(tc.tile_pool(name="sbC", bufs=1))   # persistents
    sbW = ctx.enter_context(tc.tile_pool(name="sbW", bufs=2))   # small loop tiles
    sbX = ctx.enter_context(tc.tile_pool(name="sbX", bufs=1))   # chain tensors
    sbG = ctx.enter_context(tc.tile_pool(name="sbG", bufs=3))   # gather bufs
    psA = ctx.enter_context(tc.tile_pool(name="psA", bufs=2, space="PSUM"))
    psB = ctx.enter_context(tc.tile_pool(name="psB", bufs=1, space="PSUM"))

    # ---- persistent SBUF ----
    xe = sbC.tile([128, 2 * NE], bf16, tag="xe")
    IDXW = sbC.tile([128, 10 * (HW // 16)], i16, tag="IDXW")
    womt = sbC.tile([C, 9 * 96], bf16, tag="womt")
    rlt = sbC.tile([3, NCH * 96], bf16, tag="rlt")
    r3t = sbC.tile([3, 512], bf16, tag="r3t")
    bgyt = sbC.tile([9, 1], f32, tag="bgyt")
    bgxt = sbC.tile([9, 1], f32, tag="bgxt")
    bmt = sbC.tile([9, 1], f32, tag="bmt")
    wconvt = sbC.tile([128, (NPAIR + 1) * 128], bf16, tag="wconvt")
    identt = sbC.tile([128, 128], f32, tag="identt")
    selt = sbC.tile([128, 8 * 128], f32, tag="selt")
    selbct = sbC.tile([128, 24 * 128], bf16, tag="selbct")

    for name, t in [("wom", womt), ("rl", rlt), ("r3", r3t), ("bgy", bgyt),
                    ("bgx", bgxt), ("bm", bmt), ("wconv", wconvt),
                    ("ident", identt), ("sel", selt), ("selbc", selbct)]:
        nc.sync.dma_start(out=t[:], in_=dram[name][:])

    nc.sync.dma_start(out=xe[0:64, :], in_=dram["xe"][:, :])
    nc.sync.dma_start(out=xe[64:128, :], in_=dram["xe"][:, :])
    xe3 = xe[:].rearrange("p (ph rest) -> p ph rest", ph=PH)

    # ================= per-pass: conv + chain + wrap =================
    # chain layout: quarter-group r lives at partitions [32r, 32r+9) (taps);
    # y-quantity in cols [0, QW), x-quantity in cols [QW, 2QW)
    TPP = (HW // NPASS) // 128
    SW = (HW // NPASS) // 16
    TPA = HW // 128  # all-pass transpose tiles
    NGW0 = HW // GCH
    assert (HW // NPASS) == GCH, "gw window must equal one pass's s-range"
    NGW = HW // GCH
    CPG = GCH // 512
    Sstore = {}

    def emit_preamble(ps):
        TWt = sbX.tile([128, TPP * 9 + 32], f32, tag="TWt")
        TWb = sbX.tile([128, TPP * 9 + 32], f32, tag="TWb")
        S1 = sbW.tile([128, 2 * QW], bf16, tag="S1")
        S2 = sbW.tile([128, 2 * QW], bf16, tag="S2")
        GYX2 = sbX.tile([128, 2 * QW], f32, tag="GYX2")
        M = sbX.tile([128, QW], f32, tag="M")
        nc.vector.memset(GYX2[:], 0.0)
        nc.vector.memset(M[:], 0.0)
        for cw in range(CPP):
            cg = ps * CPP + cw
            r = cg % 4
            qc = (cw // 4) * 512
            hr0 = cg * RPC
            pc = psA.tile([128, 1024], f32, tag="big", name="pcbig")[0:96, 0:512]
            for t in range(9):
                tky, tkx = t // 3, t % 3
                cb = 2 * (tkx + 1)
                rhs = xe3[0:64, hr0 + tky: hr0 + tky + RPC, cb:cb + 2 * w:2]
                nc.tensor.matmul(out=pc[:, :], lhsT=womt[:, 96 * t:96 * t + 96],
                                 rhs=rhs, start=(t == 0), stop=False)
            nc.tensor.matmul(out=pc[:, :], lhsT=rlt[:, 96 * cg:96 * cg + 96],
                             rhs=r3t[:, :], start=False, stop=True)
            nc.scalar.activation(out=GYX2[32 * r:32 * r + 9, qc:qc + 512],
                                 in_=pc[0:9, :], func=AF.Identity, bias=bgyt[:, :])
            nc.scalar.activation(out=GYX2[32 * r:32 * r + 9, QW + qc:QW + qc + 512],
                                 in_=pc[32:41, :], func=AF.Identity, bias=bgxt[:, :])
            nc.scalar.activation(out=M[32 * r:32 * r + 9, qc:qc + 512],
                                 in_=pc[64:73, :], func=AF.Sigmoid, bias=bmt[:, :])

        # ---- chain ----
        RYX2 = sbX.tile([128, 2 * QW], f32, tag="RYX2")
        TYX2 = sbX.tile([128, 2 * QW], f32, tag="TYX2")
        WYX2 = sbX.tile([128, 2 * QW], f32, tag="WYX2")
        nc.vector.tensor_scalar(out=RYX2[:], in0=GYX2[:], scalar1=MAGIC,
                                scalar2=MAGIC, op0=OP.add, op1=OP.subtract)
        nc.vector.tensor_tensor(out=TYX2[:], in0=RYX2[:], in1=GYX2[:], op=OP.is_gt)
        nc.vector.tensor_tensor(out=TYX2[:], in0=RYX2[:], in1=TYX2[:], op=OP.subtract)
        nc.vector.tensor_tensor(out=WYX2[:], in0=GYX2[:], in1=TYX2[:], op=OP.subtract)
        OMYX2 = RYX2
        nc.vector.tensor_scalar(out=OMYX2[:], in0=WYX2[:], scalar1=-1.0,
                                scalar2=1.0, op0=OP.mult, op1=OP.add)
        A = sbX.tile([128, QW], f32, tag="A")
        Bt = sbX.tile([128, QW], f32, tag="Bt")
        nc.vector.tensor_tensor(out=A[:], in0=M[:], in1=OMYX2[:, 0:QW], op=OP.mult)
        nc.vector.tensor_tensor(out=Bt[:], in0=M[:], in1=WYX2[:, 0:QW], op=OP.mult)
        s1v = S1[:, 0:2 * QW].rearrange("p (q two) -> p q two", two=2)
        s2v = S2[:, 0:2 * QW].rearrange("p (q two) -> p q two", two=2)
        nc.vector.tensor_tensor(out=s1v[:, :, 0:1], in0=A[:], in1=OMYX2[:, QW:], op=OP.mult)
        nc.vector.tensor_tensor(out=s1v[:, :, 1:2], in0=A[:], in1=WYX2[:, QW:], op=OP.mult)
        nc.vector.tensor_tensor(out=s2v[:, :, 0:1], in0=Bt[:], in1=OMYX2[:, QW:], op=OP.mult)
        nc.vector.tensor_tensor(out=s2v[:, :, 1:2], in0=Bt[:], in1=WYX2[:, QW:], op=OP.mult)
        PYX0 = WYX2
        nc.vector.tensor_scalar(out=PYX0[:, 0:QW], in0=TYX2[:, 0:QW], scalar1=1.0,
                                scalar2=0.0, op0=OP.add, op1=OP.max)
        nc.vector.tensor_scalar(out=PYX0[:, 0:QW], in0=PYX0[:, 0:QW],
                                scalar1=float(h + 1), scalar2=0.0, op0=OP.min, op1=OP.add)
        nc.vector.tensor_scalar(out=PYX0[:, QW:], in0=TYX2[:, QW:], scalar1=2.0,
                                scalar2=0.0, op0=OP.add, op1=OP.max)
        nc.vector.tensor_scalar(out=PYX0[:, QW:], in0=PYX0[:, QW:],
                                scalar1=float(w + 3), scalar2=0.0, op0=OP.min, op1=OP.add)
        PY1 = A
        nc.vector.tensor_scalar(out=PY1[:], in0=TYX2[:, 0:QW], scalar1=2.0,
                                scalar2=0.0, op0=OP.add, op1=OP.max)
        nc.vector.tensor_scalar(out=PY1[:], in0=PY1[:], scalar1=float(h + 1),
                                scalar2=0.0, op0=OP.min, op1=OP.add)
        ITOP = Bt
        IBOT = M
        nc.vector.scalar_tensor_tensor(out=ITOP[:], in0=PYX0[:, 0:QW], scalar=float(PW),
                                       in1=PYX0[:, QW:], op0=OP.mult, op1=OP.add)
        nc.vector.scalar_tensor_tensor(out=IBOT[:], in0=PY1[:], scalar=float(PW),
                                       in1=PYX0[:, QW:], op0=OP.mult, op1=OP.add)

        # ---- wrap transposes (permutes happen once, after all passes) ----
        NB = TPP // 4  # one transpose covers 4 j-blocks (one per group)
        for q0 in range(0, NB, 2):
            ptp = psA.tile([128, 1024], f32, tag="big", name="ptpbig")[:, 0:512]
            for k in range(2):
                qcbi = q0 + k
                qcb = (qcbi // 4) * 512 + (qcbi % 4) * 128
                nc.tensor.transpose(out=ptp[:, k * 256:k * 256 + 128],
                                    in_=ITOP[:, qcb:qcb + 128], identity=identt[:, :])
                nc.tensor.transpose(out=ptp[:, k * 256 + 128:k * 256 + 256],
                                    in_=IBOT[:, qcb:qcb + 128], identity=identt[:, :])
            for k in range(2):
                qcbi = q0 + k
                u, z = qcbi // 4, qcbi % 4
                for rci, TWx in ((0, TWt), (1, TWb)):
                    s0 = k * 256 + rci * 128
                    src = ptp[:, s0:s0 + 128].rearrange(
                        "p (v e) -> p v e", v=4)[:, :, 0:9]
                    base = 144 * u + 9 * z
                    dst = TWx[:, base:base + 144].rearrange(
                        "p (v x) -> p v x", v=4)[:, :, 0:9]
                    nc.vector.tensor_copy(out=dst, in_=src)

        # ---- per-pass permutes: (half, b)-outer so each selection lhsT
        # loads once and serves all 10 (pair, rc) wrap tiles ----
        pwA = psA.tile([128, 1024], f32, tag="big", name="pwA")
        pwB = psA.tile([128, 1024], f32, tag="big", name="pwB")
        for half in range(2):
            for b_ in range(8):
                lw = selt[:, 128 * b_ + 64 * half:128 * b_ + 64 * half + 64]
                for pr in range(NPAIR):
                    for rc in range(2):
                        tap = _tap_of(pr, half)
                        TWx = TWt if rc == 0 else TWb
                        rhs = TWx[:, 0:TPP * 9].rearrange(
                            "p (t e) -> p t e", e=9)[:, :, tap: tap + 1]
                        t8 = 2 * pr + rc
                        pwx, tc_ = (pwA, t8) if t8 < 8 else (pwB, t8 - 8)
                        nc.tensor.matmul(
                            out=pwx[64 * half:64 * half + 64,
                                    tc_ * 128 + b_ * TPP:tc_ * 128 + (b_ + 1) * TPP],
                            rhs=rhs, lhsT=lw,
                            start=True, stop=True, skip_group_check=True)
        for pr in range(NPAIR):
            for rc in range(2):
                t8 = 2 * pr + rc
                pwx, tc_ = (pwA, t8) if t8 < 8 else (pwB, t8 - 8)
                src = pwx[:, tc_ * 128:(tc_ + 1) * 128].rearrange(
                    "p (b t) -> p t b", b=8)
                if pr < 4:
                    db = 2 * pr * (HW // 16) + ps * 256 + rc * 128
                    nc.vector.tensor_copy(out=IDXW[:, db:db + SW], in_=src)
                else:
                    # tap8 call is half-length: groups 0-3 take positions
                    # [0,1024) (wrap slots 0-63 = t 0:8), groups 4-7 take
                    # [1024,2048) (t 8:16); top slots 0-63, bottom 64-127
                    db = 8 * (HW // 16) + ps * 256 + rc * 64
                    for hf in range(2):
                        dstq = IDXW[64 * hf:64 * hf + 64, db:db + 64].rearrange(
                            "p (t b) -> p t b", b=8)
                        nc.vector.tensor_copy(
                            out=dstq, in_=src[64 * hf:64 * hf + 64,
                                              8 * hf:8 * hf + 8, :])

        Sstore[ps] = (S1, S2)

    emit_preamble(0)
    for ps in range(NPASS):
        # ---- gather window ps; next pass's preamble is emitted after
        # pair 0 so it overlaps this window's remaining gathers ----
        gw = ps
        S1, S2 = Sstore[ps]
        pouts = {}
        for pr in range(NPAIR):
            gall = sbG.tile([128, 4 * GCH], bf16, tag="gall")
            base = 2 * pr * (HW // 16) + gw * (2 * GCH // 16)
            # tap-8 (pr 4) call is half-length: window positions are split
            # across the partition-group halves instead of duplicated
            nidx = 2 * GCH if pr < 4 else GCH
            nc.gpsimd.ap_gather(
                out_ap=gall[:, 0:2 * nidx], in_ap=xe[:],
                idxs_ap=IDXW[:, base:base + nidx // 16],
                channels=128, num_elems=NE, d=2, num_idxs=nidx)
            gtop = gall[:, 0:nidx]
            gbot = gall[:, nidx:2 * nidx]
            for ch in range(CPG):
                cg = gw * CPG + ch
                r = cg % 4
                cwp = cg % CPP
                colb = (cwp // 4) * 1024
                rowb = 9 * r + 2 * pr
                pb1 = psA.tile([128, 1024], f32, tag="big", name="pb1big")
                pb2 = psA.tile([128, 1024], f32, tag="big", name="pb2big")
                sb_blk = (4 * pr + r) if (pr < 4 or ch < 2) else (20 + r)
                selsl = selbct[:, 128 * sb_blk:128 * sb_blk + 128]
                for hb in range(2):
                    nc.tensor.matmul(out=pb1[:, hb * 512:hb * 512 + 512], lhsT=selsl,
                                     rhs=S1[0:128, colb + hb * 512:colb + hb * 512 + 512],
                                     start=True, stop=True, skip_group_check=True)
                    nc.tensor.matmul(out=pb2[:, hb * 512:hb * 512 + 512], lhsT=selsl,
                                     rhs=S2[0:128, colb + hb * 512:colb + hb * 512 + 512],
                                     start=True, stop=True, skip_group_check=True)
                sb1 = sbX.tile([128, 1024], bf16, tag="sb1")
                sb2 = sbX.tile([128, 1024], bf16, tag="sb2")
                nc.scalar.activation(out=sb1[:], in_=pb1[:], func=AF.Copy)
                nc.scalar.activation(out=sb2[:], in_=pb2[:], func=AF.Copy)
                P1 = sbX.tile([128, 1024], bf16, tag="P1")
                P2 = sbX.tile([128, 1024], bf16, tag="P2")
                if pr < 4:
                    rs = slice(0, 128)
                    gcol = ch * 1024
                else:
                    rs = slice(64 * (ch // 2), 64 * (ch // 2) + 64)
                    gcol = (ch % 2) * 1024
                nc.vector.tensor_tensor(out=P1[rs, :], in0=gtop[rs, gcol:gcol + 1024],
                                        in1=sb1[rs, :], op=OP.mult)
                nc.vector.tensor_tensor(out=P2[rs, :], in0=gbot[rs, gcol:gcol + 1024],
                                        in1=sb2[rs, :], op=OP.mult)
                if pr == 0:
                    pout_t = psB.tile([128, 512], f32, tag=f"out{ch}", name=f"pout{ch}")
                    pouts[ch] = pout_t
                pout = pouts[ch]
                p1v = P1[rs, :].rearrange("p (q two) -> p q two", two=2)
                p2v = P2[rs, :].rearrange("p (q two) -> p q two", two=2)
                if pr < 4:
                    lw = wconvt[:, 128 * pr:128 * pr + 128]
                elif ch < 2:
                    lw = wconvt[0:64, 128 * 4:128 * 5]
                else:
                    lw = wconvt[64:128, 128 * 5:128 * 6]
                for ci, rhs in enumerate([p1v[:, :, 0:1], p1v[:, :, 1:2],
                                          p2v[:, :, 0:1], p2v[:, :, 1:2]]):
                    nc.tensor.matmul(out=pout[:], lhsT=lw,
                                     rhs=rhs, start=(pr == 0 and ci == 0),
                                     stop=(pr == NPAIR - 1 and ci == 3),
                                     skip_group_check=True)
                if pr == NPAIR - 1:
                    oc = sbX.tile([128, 512], f32, tag="oc")
                    nc.vector.tensor_copy(out=oc[:], in_=pout[:])
                    nc.sync.dma_start(out=dram["out"][:, cg * 512:(cg + 1) * 512],
                                      in_=oc[:])
            if pr == 0 and ps + 1 < NPASS:
                emit_preamble(ps + 1)

    ctx.close()


def build_program(h=H, w=W, num_devices=NCORES):
    from concourse import bacc, mybir, tile

    nc = bacc.Bacc("TRN2", target_bir_lowering=False, debug=False,
                   num_devices=num_devices)
    P = _params(h, w)
    dram = {}

    def din(name, shape, np_dtype):
        dram[name] = nc.dram_tensor(name, list(shape), mybir.dt.from_np(np.dtype(np_dtype)),
                                    kind="ExternalInput").ap()

    din("xe", (C, 2 * P["NE"]), BF16)
    din("wom", (C, 9 * 96), BF16)
    din("rl", (3, P["NCH"] * 96), BF16)
    din("r3", (3, 512), BF16)
    din("bgy", (9, 1), np.float32)
    din("bgx", (9, 1), np.float32)
    din("bm", (9, 1), np.float32)
    din("wconv", (128, (NPAIR + 1) * 128), BF16)
    din("ident", (128, 128), np.float32)
    din("sel", (128, 8 * 128), np.float32)
    din("selbc", (128, 24 * 128), BF16)
    dram["out"] = nc.dram_tensor("out", [OUT, h * w], mybir.dt.float32,
                                 kind="ExternalOutput").ap()
    with tile.TileContext(nc) as tc:
        emit(nc, tc, mybir, dram, h=h, w=w)
    nc.compile()
    return nc


_CACHE = {}


def kernel(x, w_offset, b_offset, w_mask, b_mask, w_conv):
    from concourse.bass_utils import run_bass_kernel_spmd

    x = np.asarray(x)
    consts = host_consts(np.asarray(w_offset), np.asarray(b_offset),
                         np.asarray(w_mask), np.asarray(b_mask),
                         np.asarray(w_conv))
    if "nc" not in _CACHE:
        _CACHE["nc"] = build_program()
    nc = _CACHE["nc"]
    in_maps = []
    for b in range(B):
        m = {"xe": build_xe(x[b].astype(np.float32))}
        m.update(consts)
        in_maps.append(m)
    res = run_bass_kernel_spmd(nc, in_maps, list(range(NCORES)))
    out = np.stack([res.results[b]["out"].reshape(OUT, H, W) for b in range(B)])
    return out.astype(np.float32)



# revision 12
# speedup vs baseline: 3.7312x; 3.7312x over previous
"""Deformable conv (DCNv2) Bass kernel for trn2, data-parallel over batch on 8 cores.

v2: SWDGE dma_gather replaces GPSIMD ap_gather (the previous bottleneck:
~47-60us/call x 40 calls ~= 2ms serial on Pool). Per (position, tap) ONE
int16 token index is built on-device; dma_gather(transpose=True) fetches a
512B "4-corner token" (TL|TR|BL|BR x 64ch bf16) from a host-built DRAM table
and sprays it transposed into SBUF as [128 part = (corner-pair, channel),
2, num_idxs]. Pool only runs vectorized descriptor-gen (~2-3ns/idx), the 16
SDMA engines move the 75.5MB/core of corner data (~210us), and PE consumes
with K=128 matmuls (corner-pair x channel contraction).

Per-core pipeline (one batch sample per NeuronCore), NPASS passes of 2048
positions (4 quarters x 512):
  1. conv: 9 shifted matmuls + ramp matmul per 512-pos chunk on a padded
     bf16 image xs [64,(h+2)(w+2)]; ACT adds bias (+ tap offset const) and
     sigmoids the mask. Tap k lands duplicated on chain rows 32r+2k/2k+1.
  2. DVE chain: floor (magic), fracs, mask-folded row weights A=m(1-wy),
     B=m*wy, parity-selected col weight XS (even rows 1-wx, odd rows wx)
     -> W01 bf16 [A*XS | B*XS]; token idx = clamp(fy+2)*PW + clamp(fx+2).
  3. idx wrap: 4 PE transposes + DVE gather into TW[128,144], 8 permute
     matmuls (lhsT picks src partition 16u+p%16, replicating across all 8
     16-partition groups) + strided DVE int16 copies -> IDXW wrapped fmt.
  4. per tap: dma_gather (2048 idx, queue round-robin); per quarter:
     SELBK broadcast matmul (K=18) spreads (w_even,w_odd) over partition
     halves, ACT copies PSUM->bf16, DVE multiplies gathered tokens, and
     2 accumulating matmuls (lhsT = wconv[o, p%64, k]) build out[o,j] in
     PSUM over 9 taps; PSUM DMAs straight to DRAM.
Pass p+1's preamble is emitted after pass p's first gather to overlap
conv/chain/wrap with gather+consume.
"""
import sys

for _p in ("/opt/trn_rl_repo", "/opt/pypackages"):
    if _p not in sys.path:
        sys.path.append(_p)

import numpy as np
import ml_dtypes

BF16 = ml_dtypes.bfloat16

B, C, H, W = 8, 64, 128, 128
OUT, K = 128, 9
NCORES = 8
NQ = 4      # quarters (512-pos chunks) per pass
QW = 512
PPOS = NQ * QW  # positions per pass


def _params(h, w):
    hw = h * w
    d = dict(H=h, W=w, HW=hw, NPASS=hw // PPOS, NCH=hw // QW, RPC=QW // w,
             PY=h + 4, PX=w + 4, CY=h + 2, CX=w + 2)
    d["NTOK"] = d["PY"] * d["PX"]
    return d


_KY = np.repeat(np.arange(3), 3).astype(np.int64)
_KX = np.tile(np.arange(3), 3).astype(np.int64)


def sample_tensors(x, h=H, w=W):
    """Per-sample device inputs: conv image xs (bf16) + 4-corner token table."""
    P = _params(h, w)
    xp1 = np.zeros((C, h + 2, w + 2), np.float32)
    xp1[:, 1:1 + h, 1:1 + w] = x
    xs = xp1.reshape(C, -1).astype(BF16)
    # token table: t = ty*PX + tx covers corners (ty,tx),(ty,tx+1),(ty+1,tx),(ty+1,tx+1)
    # of the (2,2)-padded image; one extra zero row/col so ty=PY-1 exists.
    xp2 = np.zeros((C, h + 5, w + 5), np.float32)
    xp2[:, 2:2 + h, 2:2 + w] = x
    c00 = xp2[:, 0:P["PY"], 0:P["PX"]]
    c01 = xp2[:, 0:P["PY"], 1:P["PX"] + 1]
    c10 = xp2[:, 1:P["PY"] + 1, 0:P["PX"]]
    c11 = xp2[:, 1:P["PY"] + 1, 1:P["PX"] + 1]
    tok = np.stack([c00, c01, c10, c11], axis=0)        # [4, C, PY, PX]
    tok = tok.transpose(2, 3, 0, 1).reshape(P["NTOK"], 4 * C)
    return {"xs": xs, "tok": tok.astype(BF16)}


def host_consts(w_offset, b_offset, w_mask, b_mask, w_conv, h=H, w=W):
    P = _params(h, w)
    # conv lhsT: per conv-tap t, 96-col block; PSUM rows 0..17 = gy (tap k
    # duplicated on rows 2k,2k+1), rows 32..49 = gx, rows 64..81 = m.
    WOM = np.zeros((C, 9 * 96), np.float32)
    for t in range(9):
        for k in range(9):
            for d in range(2):
                WOM[:, 96 * t + 2 * k + d] = w_offset[2 * k, :, _KY[t], _KX[t]]
                WOM[:, 96 * t + 32 + 2 * k + d] = w_offset[2 * k + 1, :, _KY[t], _KX[t]]
                WOM[:, 96 * t + 64 + 2 * k + d] = w_mask[k, :, _KY[t], _KX[t]]
    RL = np.zeros((3, P["NCH"] * 96), np.float32)
    for c in range(P["NCH"]):
        RL[0, 96 * c: 96 * c + 18] = float(c * P["RPC"])  # gy += h0
        RL[1, 96 * c: 96 * c + 18] = 1.0                  # gy += hsub
        RL[2, 96 * c + 32: 96 * c + 50] = 1.0             # gx += wsub
    j = np.arange(QW)
    R3 = np.stack([np.ones(QW, np.float32),
                   (j // w).astype(np.float32),
                   (j % w).astype(np.float32)])
    BGY = np.repeat(b_offset[0::2] + _KY - 1.0, 2).astype(np.float32).reshape(18, 1)
    BGX = np.repeat(b_offset[1::2] + _KX - 1.0, 2).astype(np.float32).reshape(18, 1)
    BM = np.repeat(b_mask, 2).astype(np.float32).reshape(18, 1)

    wc3 = w_conv.reshape(OUT, C, 9)
    WCONV = np.zeros((128, 9 * 128), np.float32)
    for k in range(9):
        WCONV[0:64, 128 * k:128 * k + 128] = wc3[:, :, k].T
        WCONV[64:128, 128 * k:128 * k + 128] = wc3[:, :, k].T
    SELBK = np.zeros((128, 36 * 128), np.float32)
    for k in range(9):
        for r in range(4):
            b = 4 * k + r
            SELBK[32 * r + 2 * k, 128 * b:128 * b + 64] = 1.0
            SELBK[32 * r + 2 * k + 1, 128 * b + 64:128 * b + 128] = 1.0
    SEL16U = np.zeros((128, 8 * 128), np.float32)
    for u in range(8):
        for p in range(128):
            SEL16U[16 * u + p % 16, 128 * u + p] = 1.0
    IDENT = np.eye(128, dtype=np.float32)
    PARITY = (np.arange(128) % 2).astype(np.float32).reshape(128, 1)
    return {
        "wom": WOM.astype(BF16), "rl": RL.astype(BF16), "r3": R3.astype(BF16),
        "bgy": BGY, "bgx": BGX, "bm": BM,
        "wconv": WCONV.astype(BF16), "selbk": SELBK.astype(BF16),
        "sel16u": SEL16U, "ident": IDENT, "parity": PARITY,
    }


def emit(nc, tc, mybir, dram, h=H, w=W):
    P = _params(h, w)
    NPASS, NCH, RPC = P["NPASS"], P["NCH"], P["RPC"]
    CY, CX, PX = P["CY"], P["CX"], P["PX"]
    f32, bf16, i16 = mybir.dt.float32, mybir.dt.bfloat16, mybir.dt.int16
    AF = mybir.ActivationFunctionType
    OP = mybir.AluOpType
    MAGIC = 12582912.0  # 1.5 * 2^23: fp32 round-to-nearest-int trick
    NIDX = PPOS         # indices per gather call (one tap, whole pass)
    NB = QW // 128      # 128-col transpose blocks per quarter

    from contextlib import ExitStack
    ctx = ExitStack()
    sbC = ctx.enter_context(tc.tile_pool(name="sbC", bufs=1))   # persistents
    sbX = ctx.enter_context(tc.tile_pool(name="sbX", bufs=2))   # chain tensors
    sbI = ctx.enter_context(tc.tile_pool(name="sbI", bufs=2))   # idx tiles
    sbG = ctx.enter_context(tc.tile_pool(name="sbG", bufs=6))   # gather bufs
    sbS = ctx.enter_context(tc.tile_pool(name="sbS", bufs=3))   # scale/product
    psP = ctx.enter_context(tc.tile_pool(name="psP", bufs=2, space="PSUM"))
    psW = ctx.enter_context(tc.tile_pool(name="psW", bufs=2, space="PSUM"))
    psO = ctx.enter_context(tc.tile_pool(name="psO", bufs=1, space="PSUM"))

    # ---- persistent SBUF ----
    xst = sbC.tile([C, CY * CX], bf16, tag="xst")
    womt = sbC.tile([C, 9 * 96], bf16, tag="womt")
    rlt = sbC.tile([3, NCH * 96], bf16, tag="rlt")
    r3t = sbC.tile([3, QW], bf16, tag="r3t")
    bgyt = sbC.tile([18, 1], f32, tag="bgyt")
    bgxt = sbC.tile([18, 1], f32, tag="bgxt")
    bmt = sbC.tile([18, 1], f32, tag="bmt")
    wconvt = sbC.tile([128, 9 * 128], bf16, tag="wconvt")
    selbkt = sbC.tile([128, 36 * 128], bf16, tag="selbkt")
    sel16ut = sbC.tile([128, 8 * 128], f32, tag="sel16ut")
    identt = sbC.tile([128, 128], f32, tag="identt")
    part = sbC.tile([128, 1], f32, tag="part")

    for name, t in [("xs", xst), ("wom", womt), ("rl", rlt), ("r3", r3t),
                    ("bgy", bgyt), ("bgx", bgxt), ("bm", bmt),
                    ("wconv", wconvt), ("selbk", selbkt), ("sel16u", sel16ut),
                    ("ident", identt), ("parity", part)]:
        nc.sync.dma_start(out=t[:], in_=dram[name][:])
    xs3 = xst[:].rearrange("p (y x) -> p y x", y=CY)

    IDXWs = {}
    W01s = {}
    qn = [0]

    def emit_preamble(ps):
        CH = sbX.tile([128, 2 * QW], f32, tag="CH")
        M = sbX.tile([128, QW], f32, tag="M")
        nc.vector.memset(CH[:], 0.0)
        nc.vector.memset(M[:], 0.0)
        for cw in range(NQ):
            cg = ps * NQ + cw
            r = cw
            h0 = cg * RPC
            pc = psP.tile([128, QW], f32, tag="pc", name="pc")
            for t in range(9):
                tky, tkx = t // 3, t % 3
                rhs = xs3[0:C, h0 + tky: h0 + tky + RPC, tkx:tkx + w]
                nc.tensor.matmul(out=pc[0:96, :], lhsT=womt[:, 96 * t:96 * t + 96],
                                 rhs=rhs, start=(t == 0), stop=False)
            nc.tensor.matmul(out=pc[0:96, :], lhsT=rlt[:, 96 * cg:96 * cg + 96],
                             rhs=r3t[:, :], start=False, stop=True)
            nc.scalar.activation(out=CH[32 * r:32 * r + 18, 0:QW],
                                 in_=pc[0:18, :], func=AF.Identity, bias=bgyt[:, :])
            nc.scalar.activation(out=CH[32 * r:32 * r + 18, QW:2 * QW],
                                 in_=pc[32:50, :], func=AF.Identity, bias=bgxt[:, :])
            nc.scalar.activation(out=M[32 * r:32 * r + 18, :],
                                 in_=pc[64:82, :], func=AF.Sigmoid, bias=bmt[:, :])

        # ---- chain ----
        R = sbX.tile([128, 2 * QW], f32, tag="R")
        F = sbX.tile([128, 2 * QW], f32, tag="F")
        WF = sbX.tile([128, 2 * QW], f32, tag="WF")
        W01 = sbX.tile([128, 2 * QW], bf16, tag="W01")
        IDXf = sbX.tile([128, QW], f32, tag="IDXf")
        nc.vector.tensor_scalar(out=R[:], in0=CH[:], scalar1=MAGIC,
                                scalar2=MAGIC, op0=OP.add, op1=OP.subtract)
        nc.vector.tensor_tensor(out=F[:], in0=R[:], in1=CH[:], op=OP.is_gt)
        nc.vector.tensor_tensor(out=F[:], in0=R[:], in1=F[:], op=OP.subtract)
        nc.vector.tensor_tensor(out=WF[:], in0=CH[:], in1=F[:], op=OP.subtract)
        OM = R
        nc.vector.tensor_scalar(out=OM[:], in0=WF[:], scalar1=-1.0,
                                scalar2=1.0, op0=OP.mult, op1=OP.add)
        A = sbX.tile([128, QW], f32, tag="A")
        Bt = sbX.tile([128, QW], f32, tag="Bt")
        XS = sbX.tile([128, QW], f32, tag="XS")
        nc.vector.tensor_tensor(out=A[:], in0=M[:], in1=OM[:, 0:QW], op=OP.mult)
        nc.vector.tensor_tensor(out=Bt[:], in0=M[:], in1=WF[:, 0:QW], op=OP.mult)
        # XS = (1-wx) + parity*(2*wx-1): even rows 1-wx, odd rows wx
        T1 = M
        nc.vector.tensor_scalar(out=T1[:], in0=WF[:, QW:], scalar1=2.0,
                                scalar2=-1.0, op0=OP.mult, op1=OP.add)
        nc.vector.tensor_scalar(out=T1[:], in0=T1[:], scalar1=part[:, :],
                                scalar2=0.0, op0=OP.mult, op1=OP.add)
        nc.vector.tensor_tensor(out=XS[:], in0=OM[:, QW:], in1=T1[:], op=OP.add)
        nc.vector.tensor_tensor(out=W01[:, 0:QW], in0=A[:], in1=XS[:], op=OP.mult)
        nc.vector.tensor_tensor(out=W01[:, QW:], in0=Bt[:], in1=XS[:], op=OP.mult)
        # token idx = clamp(fy+2,0,h+2)*PX + clamp(fx+2,0,w+2)
        TY = A
        TX = Bt
        nc.vector.tensor_scalar(out=TY[:], in0=F[:, 0:QW], scalar1=2.0,
                                scalar2=0.0, op0=OP.add, op1=OP.max)
        nc.vector.tensor_scalar(out=TY[:], in0=TY[:], scalar1=float(h + 2),
                                scalar2=0.0, op0=OP.min, op1=OP.add)
        nc.vector.tensor_scalar(out=TX[:], in0=F[:, QW:], scalar1=2.0,
                                scalar2=0.0, op0=OP.add, op1=OP.max)
        nc.vector.tensor_scalar(out=TX[:], in0=TX[:], scalar1=float(w + 2),
                                scalar2=0.0, op0=OP.min, op1=OP.add)
        nc.vector.scalar_tensor_tensor(out=IDXf[:], in0=TY[:], scalar=float(PX),
                                       in1=TX[:], op0=OP.mult, op1=OP.add)

        # ---- idx wrap: IDXW[p, 128k+32r+8b+u] = idx(k, j=512r+128b+16u+p%16) ----
        IDXW = sbI.tile([128, 9 * (NIDX // 16)], i16, tag="IDXW")
        TW = sbX.tile([128, 144], f32, tag="TW")
        ptp = psP.tile([128, QW], f32, tag="pc", name="ptp")
        for b in range(NB):
            nc.tensor.transpose(out=ptp[:, 128 * b:128 * b + 128],
                                in_=IDXf[:, 128 * b:128 * b + 128],
                                identity=identt[:, :])
        for b in range(NB):
            src = ptp[:, 128 * b:128 * b + 128].rearrange(
                "p (r e) -> p r e", r=4)[:, :, 0:18:2]
            dst = TW[:, 36 * b:36 * b + 36].rearrange("p (r k) -> p r k", r=4)
            nc.vector.tensor_copy(out=dst, in_=src)
        pw = psP.tile([128, QW], f32, tag="pc", name="pw")
        for u in range(8):
            nc.tensor.matmul(out=pw[:, 144 * (u % 3):144 * (u % 3) + 144],
                             lhsT=sel16ut[:, 128 * u:128 * u + 128],
                             rhs=TW[:, :], start=True, stop=True,
                             skip_group_check=True)
            src = pw[:, 144 * (u % 3):144 * (u % 3) + 144].rearrange(
                "p (b r k) -> p k r b", b=4, r=4)
            dst = IDXW[:].rearrange("p (k r b u) -> p k r b u",
                                    k=9, r=4, b=4)[:, :, :, :, u:u + 1]
            nc.vector.tensor_copy(out=dst.rearrange("p k r b u -> p k r (b u)"),
                                  in_=src)
        IDXWs[ps] = IDXW
        W01s[ps] = W01

    emit_preamble(0)
    for ps in range(NPASS):
        IDXW = IDXWs.pop(ps)
        W01 = W01s.pop(ps)
        pouts = {}
        for k in range(9):
            for r in range(NQ):
                G = sbG.tile([128, 2 * QW], bf16, tag="G")
                nc.gpsimd.dma_gather(
                    out_ap=G[:].rearrange("p (e j) -> p e j", e=2),
                    in_ap=dram["tok"][:],
                    idxs_ap=IDXW[:, 128 * k + 32 * r:128 * k + 32 * r + 32],
                    num_idxs=QW, num_idxs_reg=QW, elem_size=4 * C,
                    transpose=True, queue_num=qn[0] % 4)
                qn[0] += 1
                if k == 0:
                    pouts[r] = psO.tile([128, QW], f32, tag=f"out{r}",
                                        name=f"pout{r}")
                pout = pouts[r]
                lw = wconvt[:, 128 * k:128 * k + 128]
                for e in range(2):
                    psw = psW.tile([128, QW], f32, tag="psw", name="psw")
                    sb_blk = 4 * k + r
                    nc.tensor.matmul(out=psw[:],
                                     lhsT=selbkt[:, 128 * sb_blk:128 * sb_blk + 128],
                                     rhs=W01[:, QW * e:QW * e + QW],
                                     start=True, stop=True,
                                     skip_group_check=True)
                    sbw = sbS.tile([128, QW], bf16, tag="sbw")
                    nc.scalar.activation(out=sbw[:], in_=psw[:], func=AF.Copy)
                    Pr = sbS.tile([128, QW], bf16, tag="Pr")
                    nc.vector.tensor_tensor(
                        out=Pr[:], in0=G[:, QW * e:QW * e + QW],
                        in1=sbw[:], op=OP.mult)
                    nc.tensor.matmul(out=pout[:], lhsT=lw, rhs=Pr[:],
                                     start=(k == 0 and e == 0),
                                     stop=(k == 8 and e == 1),
                                     skip_group_check=True)
                if k == 8:
                    cg = ps * NQ + r
                    oc = sbS.tile([128, QW], f32, tag="oc")
                    nc.scalar.activation(out=oc[:], in_=pout[:], func=AF.Copy)
                    nc.sync.dma_start(out=dram["out"][:, cg * QW:(cg + 1) * QW],
                                      in_=oc[:])
            if k == 0 and ps + 1 < NPASS:
                emit_preamble(ps + 1)

    ctx.close()


def build_program(h=H, w=W, num_devices=NCORES):
    from concourse import bacc, mybir, tile

    nc = bacc.Bacc("TRN2", target_bir_lowering=False, debug=False,
                   num_devices=num_devices, num_swdge_queues=4,
                   dynamic_dma_scratch_size=16384)
    P = _params(h, w)
    dram = {}

    def din(name, shape, np_dtype):
        dram[name] = nc.dram_tensor(name, list(shape),
                                    mybir.dt.from_np(np.dtype(np_dtype)),
                                    kind="ExternalInput").ap()

    din("xs", (C, P["CY"] * P["CX"]), BF16)
    din("tok", (P["NTOK"], 4 * C), BF16)
    din("wom", (C, 9 * 96), BF16)
    din("rl", (3, P["NCH"] * 96), BF16)
    din("r3", (3, QW), BF16)
    din("bgy", (18, 1), np.float32)
    din("bgx", (18, 1), np.float32)
    din("bm", (18, 1), np.float32)
    din("wconv", (128, 9 * 128), BF16)
    din("selbk", (128, 36 * 128), BF16)
    din("sel16u", (128, 8 * 128), np.float32)
    din("ident", (128, 128), np.float32)
    din("parity", (128, 1), np.float32)
    dram["out"] = nc.dram_tensor("out", [OUT, h * w], mybir.dt.float32,
                                 kind="ExternalOutput").ap()
    with tile.TileContext(nc) as tc:
        emit(nc, tc, mybir, dram, h=h, w=w)
    nc.compile()
    return nc


_CACHE = {}


def kernel(x, w_offset, b_offset, w_mask, b_mask, w_conv):
    from concourse.bass_utils import run_bass_kernel_spmd

    x = np.asarray(x)
    consts = host_consts(np.asarray(w_offset), np.asarray(b_offset),
                         np.asarray(w_mask), np.asarray(b_mask),
                         np.asarray(w_conv))
    if "nc" not in _CACHE:
        _CACHE["nc"] = build_program()
    nc = _CACHE["nc"]
    in_maps = []
    for b in range(B):
        m = dict(sample_tensors(x[b].astype(np.float32)))
        m.update(consts)
        in_maps.append(m)
    res = run_bass_kernel_spmd(nc, in_maps, list(range(NCORES)))
    out = np.stack([res.results[b]["out"].reshape(OUT, H, W) for b in range(B)])
    return out.astype(np.float32)
